# revision 19
# baseline (speedup 1.0000x reference)
"""Llama4 MoE experts kernel for 8 Trainium2 NeuronCores.

Expert-parallel: tokens are pre-sorted per expert (8192 tokens = 8 experts
x 1024 tokens), so core e gets expert e's tokens + weights and computes
   out_e = (up_e * silu(gate_e)) @ W2_e,   [gate_e|up_e] = x_e @ W1_e
entirely locally (no collectives).

Mixed precision: the base path runs in fp16 (same PE rate as bf16, ~8x
less rounding error), and the last 512 intermediate channels run their
gate/up projections in fp8-e4m3 using the PE's DoubleRow mode (2
contraction rows/cycle, 2x matmul throughput). Their SwiGLU output is
stored back to fp16, so the down projection is one uniform full-width
fp16 chain. Measured end-to-end relative error is ~1.9% (gate: 2e-2);
the fp8 gate/up work is halved, saving ~27us of tensor-engine time vs
an all-bf16 kernel. Global absmax scales are computed host-side from
the actual inputs and folded into on-device activation scales.
"""

import numpy as np
import ml_dtypes

E, T, H, F, P = 8, 1024, 2048, 4096, 128
KH, KF = H // P, F // P          # 16 k-blocks over H, 32 over F
F8 = 512                         # fp8 gate/up channels (last 4 f-blocks)
NB8 = F8 // P                    # 4 fp8 f-blocks
KFB = (F - F8) // P              # 28 fp16 f-blocks
CB = 2 * KFB                     # 56 fp16 column blocks of W1 (gate, then up)
HB = H // 256                    # 8 output-column blocks of 256
MH = H // 256                    # 8 DoubleRow k-tiles over H (128 pairs each)

_CACHE = {}


def _build():
    import concourse.bacc as bacc
    import concourse.tile as tile
    import concourse.mybir as mybir

    fp16 = mybir.dt.float16
    bf16 = mybir.dt.bfloat16
    f32 = mybir.dt.float32
    f8 = mybir.dt.float8e4
    DR = mybir.MatmulPerfMode.DoubleRow

    sx, sw1g, sw1u = _CACHE["scales"]
    silu_scale = float(1.0 / (sx * sw1g))
    up_scale = float(1.0 / (sx * sw1u))

    nc = bacc.Bacc("TRN2", target_bir_lowering=False, debug=False, num_devices=E)

    # xt and the first W1 tile pair are bf16 (same PE rate, half the bytes):
    # they gate the DMA ramp before the first matmul chains can run, and the
    # PE converts each operand's dtype independently on load/stream.
    xt_d = nc.dram_tensor("xt", [P, KH, T], bf16, kind="ExternalInput").ap()
    w1f_d = nc.dram_tensor("w1f", [2, P, KH, P], bf16, kind="ExternalInput").ap()
    w1_d = nc.dram_tensor("w1", [CB, P, KH, P], fp16, kind="ExternalInput").ap()
    w2_d = nc.dram_tensor("w2", [HB, P, KF, 256], fp16, kind="ExternalInput").ap()
    x8_d = nc.dram_tensor("x8", [P, MH, 2, T], f8, kind="ExternalInput").ap()
    w1g8_d = nc.dram_tensor("w1g8", [NB8, P, MH, 2, P], f8, kind="ExternalInput").ap()
    w1u8_d = nc.dram_tensor("w1u8", [NB8, P, MH, 2, P], f8, kind="ExternalInput").ap()
    out_d = nc.dram_tensor("out", [T, H], fp16, kind="ExternalOutput").ap()

    with tile.TileContext(nc) as tc:
        with (
            tc.tile_pool(name="resident", bufs=1) as res_pool,
            tc.tile_pool(name="w1pool", bufs=3) as w1_pool,
            tc.tile_pool(name="w2pool", bufs=2) as w2_pool,
            tc.tile_pool(name="tmppool", bufs=3) as tmp_pool,
            tc.tile_pool(name="outpool", bufs=4) as out_pool,
            tc.tile_pool(name="psg", bufs=2, space="PSUM") as psg_pool,
            tc.tile_pool(name="psu", bufs=2, space="PSUM") as psu_pool,
            tc.tile_pool(name="pso", bufs=4, space="PSUM") as pso_pool,
        ):
            xT = res_pool.tile([P, KH, T], bf16, name="xT")
            interT = res_pool.tile([P, KF, T], fp16, name="interT")
            x8t = res_pool.tile([P, MH, 2, T], f8, name="x8t")
            w1g8t = res_pool.tile([P, NB8, MH, 2, P], f8, name="w1g8t")
            w1u8t = res_pool.tile([P, NB8, MH, 2, P], f8, name="w1u8t")

            # Phase 1a (fp16 blocks): gate/up projections + SwiGLU -> interT.
            # DMA order matters for ramp-up: the first matmul chain needs
            # xT[:, 0] + w1g_0, so interleave the W1 i=0 tiles right after
            # the first xT block instead of queueing all of xT first.
            for i in range(KFB):
                dt_i = bf16 if i == 0 else fp16
                w1g = w1_pool.tile([P, KH, P], dt_i, tag="w1g", name=f"w1g_{i}")
                w1u = w1_pool.tile([P, KH, P], dt_i, tag="w1u", name=f"w1u_{i}")
                if i == 0:
                    # first-use order: the gate chain consumes xT[kb] every
                    # 213ns but w1u isn't needed until the chain ends, so
                    # slot w1u_0 after xT[1..2] instead of ahead of them
                    nc.sync.dma_start(out=xT[:, 0, :], in_=xt_d[:, 0, :])
                    nc.sync.dma_start(out=w1g[:], in_=w1f_d[0])
                    nc.sync.dma_start(out=xT[:, 1, :], in_=xt_d[:, 1, :])
                    nc.sync.dma_start(out=xT[:, 2, :], in_=xt_d[:, 2, :])
                    nc.sync.dma_start(out=w1u[:], in_=w1f_d[1])
                    for kb in range(3, KH):
                        nc.sync.dma_start(out=xT[:, kb, :], in_=xt_d[:, kb, :])
                else:
                    nc.sync.dma_start(out=w1g[:], in_=w1_d[i])
                    nc.sync.dma_start(out=w1u[:], in_=w1_d[KFB + i])
                if i == 2:
                    # fp8 operands are small; stream them in early, long
                    # before the fp8 chains at the end of phase 1 need them
                    nc.sync.dma_start(out=x8t[:], in_=x8_d[:])
                    for cb2 in range(NB8):
                        nc.sync.dma_start(out=w1g8t[:, cb2], in_=w1g8_d[cb2])
                        nc.sync.dma_start(out=w1u8t[:, cb2], in_=w1u8_d[cb2])
                for th in range(2):
                    ts_ = slice(th * 512, (th + 1) * 512)
                    pg = psg_pool.tile([P, 512], f32, tag="pg", name=f"pg_{i}_{th}")
                    pu = psu_pool.tile([P, 512], f32, tag="pu", name=f"pu_{i}_{th}")
                    for kb in range(KH):
                        nc.tensor.matmul(
                            pg[:], lhsT=w1g[:, kb, :], rhs=xT[:, kb, ts_],
                            start=(kb == 0), stop=(kb == KH - 1),
                        )
                    for kb in range(KH):
                        nc.tensor.matmul(
                            pu[:], lhsT=w1u[:, kb, :], rhs=xT[:, kb, ts_],
                            start=(kb == 0), stop=(kb == KH - 1),
                        )
                    sg = tmp_pool.tile([P, 512], f32, tag="sg", name=f"sg_{i}_{th}")
                    nc.scalar.activation(
                        sg[:], pg[:], mybir.ActivationFunctionType.Silu
                    )
                    nc.vector.tensor_mul(interT[:, i, ts_], sg[:], pu[:])

            # Phase 1b (fp8 blocks): DoubleRow packs 2 contraction rows per
            # partition, so 8 k-tiles cover H=2048 at 2 MACs/cell/cycle.
            # PSUM holds sx*sw1g*gate; the activation scale rescales to true
            # units before silu, the up path likewise, and the DVE product
            # lands in interT as fp16 so phase 2 stays uniform.
            for cb2 in range(NB8):
                for th in range(2):
                    ts_ = slice(th * 512, (th + 1) * 512)
                    pg8 = psg_pool.tile(
                        [P, 512], f32, tag="pg", name=f"pg8_{cb2}_{th}"
                    )
                    pu8 = psu_pool.tile(
                        [P, 512], f32, tag="pu", name=f"pu8_{cb2}_{th}"
                    )
                    for m in range(MH):
                        nc.tensor.matmul(
                            pg8[:], lhsT=w1g8t[:, cb2, m, :, :],
                            rhs=x8t[:, m, :, ts_],
                            start=(m == 0), stop=(m == MH - 1), perf_mode=DR,
                        )
                    for m in range(MH):
                        nc.tensor.matmul(
                            pu8[:], lhsT=w1u8t[:, cb2, m, :, :],
                            rhs=x8t[:, m, :, ts_],
                            start=(m == 0), stop=(m == MH - 1), perf_mode=DR,
                        )
                    sg8 = tmp_pool.tile(
                        [P, 512], f32, tag="sg", name=f"sg8_{cb2}_{th}"
                    )
                    nc.scalar.activation(
                        sg8[:], pg8[:], mybir.ActivationFunctionType.Silu,
                        scale=silu_scale,
                    )
                    us8 = tmp_pool.tile(
                        [P, 512], f32, tag="sg", name=f"us8_{cb2}_{th}"
                    )
                    nc.scalar.mul(us8[:], pu8[:], up_scale)
                    nc.vector.tensor_mul(
                        interT[:, KFB + cb2, ts_], sg8[:], us8[:]
                    )

            # Phase 2: down projection, one uniform fp16 chain over all 32
            # k-blocks, streaming W2 once.
            for hb in range(HB):
                w2t = w2_pool.tile([P, KF, 256], fp16, tag="w2", name=f"w2_{hb}")
                nc.sync.dma_start(out=w2t[:], in_=w2_d[hb])
                for tb in range(T // P):
                    tbs = slice(tb * P, (tb + 1) * P)
                    po = pso_pool.tile([P, 256], f32, tag="po", name=f"po_{hb}_{tb}")
                    for kb in range(KF):
                        nc.tensor.matmul(
                            po[:],
                            lhsT=interT[:, kb, tbs],
                            rhs=w2t[:, kb, :],
                            start=(kb == 0), stop=(kb == KF - 1),
                        )
                    ob = out_pool.tile([P, 256], fp16, tag="ob", name=f"ob_{hb}_{tb}")
                    nc.scalar.copy(ob[:], po[:])
                    nc.sync.dma_start(
                        out=out_d[tbs, hb * 256:(hb + 1) * 256],
                        in_=ob[:],
                    )

    nc.compile()
    return nc


def _prep_inputs(hidden_states, gate_up_proj, down_proj):
    f8 = ml_dtypes.float8_e4m3
    FB = F - F8
    xr = np.asarray(hidden_states, np.float32).reshape(E, T, H)
    W1 = np.asarray(gate_up_proj, np.float32)
    W2 = np.asarray(down_proj, np.float32)
    w1g8_cols = W1[:, :, FB:F]
    w1u8_cols = W1[:, :, F + FB:]

    # global absmax scales for the fp8 path (baked into the program)
    sx = 240.0 / np.abs(xr).max()
    sw1g = 240.0 / np.abs(w1g8_cols).max()
    sw1u = 240.0 / np.abs(w1u8_cols).max()
    scales = (float(sx), float(sw1g), float(sw1u))
    if _CACHE.get("scales") != scales:
        # scales are baked into the compiled program; rebuild on new inputs
        _CACHE.pop("nc", None)
    _CACHE["scales"] = scales

    def q8(a, s):
        return np.asarray(np.clip(a * s, -240.0, 240.0), f8)

    # xt[e, p, k, t] = x[e, t, k*128+p], bf16 to halve the DMA ramp
    xt = xr.transpose(0, 2, 1).reshape(E, KH, P, T).transpose(0, 2, 1, 3)
    xt = np.ascontiguousarray(xt).astype(ml_dtypes.bfloat16)
    # w1b: fp16 gate blocks 0..27 then up blocks 0..27 (of the 64-block grid)
    w1b = W1.reshape(E, KH, P, 2 * KF, P)
    w1b = w1b[:, :, :, list(range(KFB)) + list(range(KF, KF + KFB)), :]
    w1bt = w1b.transpose(0, 3, 2, 1, 4)
    # block 0 of gate and up also as bf16 for the ramp-critical first tiles
    w1f = np.ascontiguousarray(
        w1bt[:, [0, KFB], :, :, :]
    ).astype(ml_dtypes.bfloat16)
    w1b = np.ascontiguousarray(w1bt).astype(np.float16)
    # w2b[e, hb, p, kb, j] = W2[e, kb*128+p, hb*256+j]
    w2b = W2.reshape(E, KF, P, HB, 256)
    w2b = np.ascontiguousarray(w2b.transpose(0, 3, 2, 1, 4)).astype(np.float16)
    # x8[e, p, m, j, t] = q8(x)[e, t, 256m+128j+p]
    x8 = q8(xr, sx).reshape(E, T, MH, 2, P).transpose(0, 4, 2, 3, 1)
    x8 = np.ascontiguousarray(x8)
    # w1g8[e, cb2, p, m, j, c] = q8(W1g fp8 cols)[e, 256m+128j+p, 128*cb2+c]
    w1g8 = q8(w1g8_cols, sw1g).reshape(E, MH, 2, P, NB8, P)
    w1g8 = np.ascontiguousarray(w1g8.transpose(0, 4, 3, 1, 2, 5))
    w1u8 = q8(w1u8_cols, sw1u).reshape(E, MH, 2, P, NB8, P)
    w1u8 = np.ascontiguousarray(w1u8.transpose(0, 4, 3, 1, 2, 5))
    return [
        {"xt": np.ascontiguousarray(xt[e]),
         "w1f": np.ascontiguousarray(w1f[e]),
         "w1": np.ascontiguousarray(w1b[e]),
         "w2": np.ascontiguousarray(w2b[e]),
         "x8": x8[e],
         "w1g8": w1g8[e],
         "w1u8": w1u8[e]}
        for e in range(E)
    ]


def run_spmd(in_maps, trace=False, trace_kwargs=None):
    from concourse.bass_utils import run_bass_kernel_spmd
    from concourse.bass_interp import get_hw_module

    if "nc" not in _CACHE:
        _CACHE["nc"] = _build()
    nc = _CACHE["nc"]

    old_m = nc.m
    nc.m = get_hw_module(nc.m)
    try:
        res = run_bass_kernel_spmd(
            nc, in_maps, core_ids=list(range(E)),
            trace=trace, **(trace_kwargs or {}),
        )
    finally:
        nc.m = old_m
    return res


def kernel(hidden_states, gate_up_proj, down_proj):
    in_maps = _prep_inputs(hidden_states, gate_up_proj, down_proj)
    res = run_spmd(in_maps)
    out = np.concatenate(
        [np.asarray(res.results[e]["out"]) for e in range(E)], axis=0
    )
    return out.astype(np.float32)


# revision 20
# speedup vs baseline: 1.0052x; 1.0052x over previous
"""Llama4 MoE experts kernel for 8 Trainium2 NeuronCores.

Expert-parallel: tokens are pre-sorted per expert (8192 tokens = 8 experts
x 1024 tokens), so core e gets expert e's tokens + weights and computes
   out_e = (up_e * silu(gate_e)) @ W2_e,   [gate_e|up_e] = x_e @ W1_e
entirely locally (no collectives).

Mixed precision: the base path runs in fp16 (same PE rate as bf16, ~8x
less rounding error), and the last 512 intermediate channels run their
gate/up projections in fp8-e4m3 using the PE's DoubleRow mode (2
contraction rows/cycle, 2x matmul throughput). Their SwiGLU output is
stored back to fp16, so the down projection is one uniform full-width
fp16 chain. Measured end-to-end relative error is ~1.9% (gate: 2e-2);
the fp8 gate/up work is halved, saving ~27us of tensor-engine time vs
an all-bf16 kernel. Global absmax scales are computed host-side from
the actual inputs and folded into on-device activation scales.
"""

import numpy as np
import ml_dtypes

E, T, H, F, P = 8, 1024, 2048, 4096, 128
KH, KF = H // P, F // P          # 16 k-blocks over H, 32 over F
F8 = 512                         # fp8 gate/up channels (last 4 f-blocks)
NB8 = F8 // P                    # 4 fp8 f-blocks
KFB = (F - F8) // P              # 28 fp16 f-blocks
CB = 2 * KFB                     # 56 fp16 column blocks of W1 (gate, then up)
HB = H // 256                    # 8 output-column blocks of 256
MH = H // 256                    # 8 DoubleRow k-tiles over H (128 pairs each)

_CACHE = {}


def _build():
    import concourse.bacc as bacc
    import concourse.tile as tile
    import concourse.mybir as mybir

    fp16 = mybir.dt.float16
    f32 = mybir.dt.float32
    f8 = mybir.dt.float8e4
    DR = mybir.MatmulPerfMode.DoubleRow

    sx, sw1g, sw1u = _CACHE["scales"]
    silu_scale = float(1.0 / (sx * sw1g))
    up_scale = float(1.0 / (sx * sw1u))

    nc = bacc.Bacc("TRN2", target_bir_lowering=False, debug=False, num_devices=E)

    xt_d = nc.dram_tensor("xt", [P, KH, T], fp16, kind="ExternalInput").ap()
    w1_d = nc.dram_tensor("w1", [CB, P, KH, P], fp16, kind="ExternalInput").ap()
    w2_d = nc.dram_tensor("w2", [HB, P, KF, 256], fp16, kind="ExternalInput").ap()
    x8_d = nc.dram_tensor("x8", [P, MH, 2, T], f8, kind="ExternalInput").ap()
    w1g8_d = nc.dram_tensor("w1g8", [NB8, P, MH, 2, P], f8, kind="ExternalInput").ap()
    w1u8_d = nc.dram_tensor("w1u8", [NB8, P, MH, 2, P], f8, kind="ExternalInput").ap()
    out_d = nc.dram_tensor("out", [T, H], fp16, kind="ExternalOutput").ap()

    with tile.TileContext(nc) as tc:
        with (
            tc.tile_pool(name="resident", bufs=1) as res_pool,
            tc.tile_pool(name="w1pool", bufs=3) as w1_pool,
            tc.tile_pool(name="w2pool", bufs=2) as w2_pool,
            tc.tile_pool(name="tmppool", bufs=3) as tmp_pool,
            tc.tile_pool(name="outpool", bufs=4) as out_pool,
            tc.tile_pool(name="psg", bufs=2, space="PSUM") as psg_pool,
            tc.tile_pool(name="psu", bufs=2, space="PSUM") as psu_pool,
            tc.tile_pool(name="pso", bufs=4, space="PSUM") as pso_pool,
        ):
            xT = res_pool.tile([P, KH, T], fp16, name="xT")
            interT = res_pool.tile([P, KF, T], fp16, name="interT")
            x8t = res_pool.tile([P, MH, 2, T], f8, name="x8t")
            w1g8t = res_pool.tile([P, NB8, MH, 2, P], f8, name="w1g8t")
            w1u8t = res_pool.tile([P, NB8, MH, 2, P], f8, name="w1u8t")

            # Phase 1a (fp16 blocks): gate/up projections + SwiGLU -> interT.
            # DMA order matters for ramp-up: the first matmul chain needs
            # xT[:, 0] + w1g_0, so interleave the W1 i=0 tiles right after
            # the first xT block instead of queueing all of xT first.
            for i in range(KFB):
                w1g = w1_pool.tile([P, KH, P], fp16, tag="w1g", name=f"w1g_{i}")
                w1u = w1_pool.tile([P, KH, P], fp16, tag="w1u", name=f"w1u_{i}")
                if i == 0:
                    # first-use order: the gate chain consumes xT[kb] every
                    # 213ns but w1u isn't needed until the chain ends, so
                    # slot w1u_0 after xT[1..2] instead of ahead of them
                    nc.sync.dma_start(out=xT[:, 0, :], in_=xt_d[:, 0, :])
                    nc.sync.dma_start(out=w1g[:], in_=w1_d[i])
                    nc.sync.dma_start(out=xT[:, 1, :], in_=xt_d[:, 1, :])
                    nc.sync.dma_start(out=xT[:, 2, :], in_=xt_d[:, 2, :])
                    nc.sync.dma_start(out=w1u[:], in_=w1_d[KFB + i])
                    for kb in range(3, KH):
                        nc.sync.dma_start(out=xT[:, kb, :], in_=xt_d[:, kb, :])
                else:
                    nc.sync.dma_start(out=w1g[:], in_=w1_d[i])
                    nc.sync.dma_start(out=w1u[:], in_=w1_d[KFB + i])
                if i == 2:
                    # fp8 operands are small; stream them in early, long
                    # before the fp8 chains at the end of phase 1 need them
                    nc.sync.dma_start(out=x8t[:], in_=x8_d[:])
                    for cb2 in range(NB8):
                        nc.sync.dma_start(out=w1g8t[:, cb2], in_=w1g8_d[cb2])
                        nc.sync.dma_start(out=w1u8t[:, cb2], in_=w1u8_d[cb2])
                for th in range(2):
                    ts_ = slice(th * 512, (th + 1) * 512)
                    pg = psg_pool.tile([P, 512], f32, tag="pg", name=f"pg_{i}_{th}")
                    pu = psu_pool.tile([P, 512], f32, tag="pu", name=f"pu_{i}_{th}")
                    for kb in range(KH):
                        nc.tensor.matmul(
                            pg[:], lhsT=w1g[:, kb, :], rhs=xT[:, kb, ts_],
                            start=(kb == 0), stop=(kb == KH - 1),
                        )
                    for kb in range(KH):
                        nc.tensor.matmul(
                            pu[:], lhsT=w1u[:, kb, :], rhs=xT[:, kb, ts_],
                            start=(kb == 0), stop=(kb == KH - 1),
                        )
                    sg = tmp_pool.tile([P, 512], f32, tag="sg", name=f"sg_{i}_{th}")
                    nc.scalar.activation(
                        sg[:], pg[:], mybir.ActivationFunctionType.Silu
                    )
                    nc.vector.tensor_mul(interT[:, i, ts_], sg[:], pu[:])

            # Phase 1b (fp8 blocks): DoubleRow packs 2 contraction rows per
            # partition, so 8 k-tiles cover H=2048 at 2 MACs/cell/cycle.
            # PSUM holds sx*sw1g*gate; the activation scale rescales to true
            # units before silu, the up path likewise, and the DVE product
            # lands in interT as fp16 so phase 2 stays uniform.
            for cb2 in range(NB8):
                for th in range(2):
                    ts_ = slice(th * 512, (th + 1) * 512)
                    pg8 = psg_pool.tile(
                        [P, 512], f32, tag="pg", name=f"pg8_{cb2}_{th}"
                    )
                    pu8 = psu_pool.tile(
                        [P, 512], f32, tag="pu", name=f"pu8_{cb2}_{th}"
                    )
                    for m in range(MH):
                        nc.tensor.matmul(
                            pg8[:], lhsT=w1g8t[:, cb2, m, :, :],
                            rhs=x8t[:, m, :, ts_],
                            start=(m == 0), stop=(m == MH - 1), perf_mode=DR,
                        )
                    for m in range(MH):
                        nc.tensor.matmul(
                            pu8[:], lhsT=w1u8t[:, cb2, m, :, :],
                            rhs=x8t[:, m, :, ts_],
                            start=(m == 0), stop=(m == MH - 1), perf_mode=DR,
                        )
                    sg8 = tmp_pool.tile(
                        [P, 512], f32, tag="sg", name=f"sg8_{cb2}_{th}"
                    )
                    nc.scalar.activation(
                        sg8[:], pg8[:], mybir.ActivationFunctionType.Silu,
                        scale=silu_scale,
                    )
                    us8 = tmp_pool.tile(
                        [P, 512], f32, tag="sg", name=f"us8_{cb2}_{th}"
                    )
                    nc.scalar.mul(us8[:], pu8[:], up_scale)
                    nc.vector.tensor_mul(
                        interT[:, KFB + cb2, ts_], sg8[:], us8[:]
                    )

            # Phase 2: down projection, one uniform fp16 chain over all 32
            # k-blocks, streaming W2 once.
            for hb in range(HB):
                w2t = w2_pool.tile([P, KF, 256], fp16, tag="w2", name=f"w2_{hb}")
                nc.sync.dma_start(out=w2t[:], in_=w2_d[hb])
                for tb in range(T // P):
                    tbs = slice(tb * P, (tb + 1) * P)
                    po = pso_pool.tile([P, 256], f32, tag="po", name=f"po_{hb}_{tb}")
                    for kb in range(KF):
                        nc.tensor.matmul(
                            po[:],
                            lhsT=interT[:, kb, tbs],
                            rhs=w2t[:, kb, :],
                            start=(kb == 0), stop=(kb == KF - 1),
                        )
                    ob = out_pool.tile([P, 256], fp16, tag="ob", name=f"ob_{hb}_{tb}")
                    nc.scalar.copy(ob[:], po[:])
                    nc.sync.dma_start(
                        out=out_d[tbs, hb * 256:(hb + 1) * 256],
                        in_=ob[:],
                    )

    nc.compile()
    return nc


def _prep_inputs(hidden_states, gate_up_proj, down_proj):
    f8 = ml_dtypes.float8_e4m3
    FB = F - F8
    xr = np.asarray(hidden_states, np.float32).reshape(E, T, H)
    W1 = np.asarray(gate_up_proj, np.float32)
    W2 = np.asarray(down_proj, np.float32)
    w1g8_cols = W1[:, :, FB:F]
    w1u8_cols = W1[:, :, F + FB:]

    # global absmax scales for the fp8 path (baked into the program)
    sx = 240.0 / np.abs(xr).max()
    sw1g = 240.0 / np.abs(w1g8_cols).max()
    sw1u = 240.0 / np.abs(w1u8_cols).max()
    scales = (float(sx), float(sw1g), float(sw1u))
    if _CACHE.get("scales") != scales:
        # scales are baked into the compiled program; rebuild on new inputs
        _CACHE.pop("nc", None)
    _CACHE["scales"] = scales

    def q8(a, s):
        return np.asarray(np.clip(a * s, -240.0, 240.0), f8)

    # xt[e, p, k, t] = x[e, t, k*128+p]
    xt = xr.transpose(0, 2, 1).reshape(E, KH, P, T).transpose(0, 2, 1, 3)
    xt = np.ascontiguousarray(xt).astype(np.float16)
    # w1b: fp16 gate blocks 0..27 then up blocks 0..27 (of the 64-block grid)
    w1b = W1.reshape(E, KH, P, 2 * KF, P)
    w1b = w1b[:, :, :, list(range(KFB)) + list(range(KF, KF + KFB)), :]
    w1b = np.ascontiguousarray(w1b.transpose(0, 3, 2, 1, 4)).astype(np.float16)
    # w2b[e, hb, p, kb, j] = W2[e, kb*128+p, hb*256+j]
    w2b = W2.reshape(E, KF, P, HB, 256)
    w2b = np.ascontiguousarray(w2b.transpose(0, 3, 2, 1, 4)).astype(np.float16)
    # x8[e, p, m, j, t] = q8(x)[e, t, 256m+128j+p]
    x8 = q8(xr, sx).reshape(E, T, MH, 2, P).transpose(0, 4, 2, 3, 1)
    x8 = np.ascontiguousarray(x8)
    # w1g8[e, cb2, p, m, j, c] = q8(W1g fp8 cols)[e, 256m+128j+p, 128*cb2+c]
    w1g8 = q8(w1g8_cols, sw1g).reshape(E, MH, 2, P, NB8, P)
    w1g8 = np.ascontiguousarray(w1g8.transpose(0, 4, 3, 1, 2, 5))
    w1u8 = q8(w1u8_cols, sw1u).reshape(E, MH, 2, P, NB8, P)
    w1u8 = np.ascontiguousarray(w1u8.transpose(0, 4, 3, 1, 2, 5))
    return [
        {"xt": np.ascontiguousarray(xt[e]),
         "w1": np.ascontiguousarray(w1b[e]),
         "w2": np.ascontiguousarray(w2b[e]),
         "x8": x8[e],
         "w1g8": w1g8[e],
         "w1u8": w1u8[e]}
        for e in range(E)
    ]


def run_spmd(in_maps, trace=False, trace_kwargs=None):
    from concourse.bass_utils import run_bass_kernel_spmd
    from concourse.bass_interp import get_hw_module

    if "nc" not in _CACHE:
        _CACHE["nc"] = _build()
    nc = _CACHE["nc"]

    old_m = nc.m
    nc.m = get_hw_module(nc.m)
    try:
        res = run_bass_kernel_spmd(
            nc, in_maps, core_ids=list(range(E)),
            trace=trace, **(trace_kwargs or {}),
        )
    finally:
        nc.m = old_m
    return res


def kernel(hidden_states, gate_up_proj, down_proj):
    in_maps = _prep_inputs(hidden_states, gate_up_proj, down_proj)
    res = run_spmd(in_maps)
    out = np.concatenate(
        [np.asarray(res.results[e]["out"]) for e in range(E)], axis=0
    )
    return out.astype(np.float32)
